# revision 2
# baseline (speedup 1.0000x reference)
"""Trainium2 Bass kernel v2 for nn_EnhancedGNN (3-layer GCN + mean-pool + FC).

Design (dst-sharded, zT-oriented, host-baked normalization):
  - Tables are packed bf16 rows [node, 64] (128B). Gathers read 256B
    granules (2 packed rows) at stride 512B, giving 4 parity classes
    p = src % 4 with int16 granule index src // 4 (< 25088).
  - Slots are laid out uniformly across cores: per (window, parity)
    segment size = max over cores; per group (7 windows) one gather call
    per parity class, tail-padded with -1 (stripped, free).
  - Scatter = PE matmuls: out zT[f, 128 dst] += M_b.T @ C_tile, where
    M_b = gathered rows (lhsT, stationary) and C_tile [128, 128] bf16 is
    HOST-BAKED (value norm_e = dinv[src] w dinv[dst] at [slot, dstrel]).
    Self-loops are per-window diag(dinv^2) tiles against the resident
    own-table slice. deg/dinv all precomputed on host.
  - Epilogue per window works on zT [F, 128] in PSUM: W-matmuls with
    stationary weights, ACT relu+bias, PE transpose to row-major,
    write-back to the next layer's table; AllGather per layer; L3 pools
    via batch one-hot matmul; tiny AllReduce; FC on every core.
"""

import os
import sys
import types

import numpy as np

try:
    import ml_dtypes
    BF16 = ml_dtypes.bfloat16
except Exception:  # pragma: no cover
    BF16 = np.float32

# ---------------------------------------------------------------- constants
N_NODES = 100000
F_IN = 16
N_GRAPHS = 64
P = 128
N_CORES = 8
W_PER_CORE = 98
NPC = W_PER_CORE * P                 # 12544
NODES_PAD = N_CORES * NPC            # 100352
NPAR = 4                             # parity classes (stride 512B)
NGRAN = NODES_PAD // NPAR            # 25088 granules of 512B
WG = 7                               # windows per gather group
N_GROUPS = W_PER_CORE // WG          # 14
SPLIT_W = 70                         # windows in AllGather part A
SPLIT_ROWS = SPLIT_W * P             # 8960
FD = 64                              # table row payload (bf16)

LAST_EXEC_TIME_NS = None
LAST_TRACE = None
LAST_RESULT = None


# ---------------------------------------------------------------- host prep
def _prep(src, dst, w):
    """Builds uniform slot/tile structure + per-core data arrays."""
    E = src.shape[0]
    deg = np.bincount(dst, weights=w.astype(np.float64), minlength=N_NODES)
    deg += 1.0
    dinv = (1.0 / np.sqrt(deg)).astype(np.float32)
    dinv_pad = np.ones(NODES_PAD, np.float32)
    dinv_pad[:N_NODES] = dinv
    norm = dinv[src] * w.astype(np.float32) * dinv[dst]

    core = dst // NPC
    wl = (dst % NPC) // P
    dstrel = dst % P
    par = (src % NPAR).astype(np.int64)
    gidx = (src // NPAR).astype(np.int64)

    order = np.lexsort((src, par, wl, core))
    oc = core[order]
    owl = wl[order]
    opar = par[order]
    ogidx = gidx[order]
    odst = dstrel[order]
    onorm = norm[order]

    # per (core, w, p) counts
    key = (oc * W_PER_CORE + owl) * NPAR + opar
    cnt = np.bincount(key, minlength=N_CORES * W_PER_CORE * NPAR)
    cnt = cnt.reshape(N_CORES, W_PER_CORE, NPAR)
    segsz = cnt.max(axis=0)                      # [98, 4] uniform
    seg_start_edge = np.zeros((N_CORES, W_PER_CORE, NPAR), np.int64)
    flat = cnt.reshape(-1)
    seg_start_edge.reshape(-1)[1:] = np.cumsum(flat)[:-1]

    # ---- uniform slot layout -------------------------------------------
    # group g covers windows [g*WG, (g+1)*WG); call (g, p) concatenates
    # the group's p-segments, tail-padded to a block (128) multiple.
    seg_slot = np.zeros((W_PER_CORE, NPAR), np.int64)   # global slot of seg
    call_nblk = np.zeros((N_GROUPS, NPAR), np.int64)
    call_len = np.zeros((N_GROUPS, NPAR), np.int64)     # real idx count
    call_blk0 = np.zeros((N_GROUPS, NPAR), np.int64)    # first global block
    blk = 0
    for g in range(N_GROUPS):
        for p in range(NPAR):
            pos = 0
            for wi in range(g * WG, (g + 1) * WG):
                seg_slot[wi, p] = blk * P + pos
                pos += segsz[wi, p]
            call_len[g, p] = pos
            nb = -(-pos // P)
            call_nblk[g, p] = nb
            call_blk0[g, p] = blk
            blk += nb
    NBLK = blk
    NSLOT = NBLK * P

    # ---- tile structure (uniform) --------------------------------------
    # consumption: per window: [self-loop] + per parity: blocks jlo..jhi
    tile_of = {}                 # (w, p, jglobal) -> tile id
    win_tiles = []               # per window: list of (kind, p, jglobal)
    tid = 0
    for wi in range(W_PER_CORE):
        tl = [("loop", 0, 0)]
        tid += 1
        for p in range(NPAR):
            s0 = seg_slot[wi, p]
            c = segsz[wi, p]
            if c == 0:
                continue
            jlo, jhi = s0 // P, (s0 + c - 1) // P
            for j in range(jlo, jhi + 1):
                tile_of[(wi, p, j)] = tid
                tl.append(("seg", p, j))
                tid += 1
        win_tiles.append(tl)
    NTILES = tid
    win_tile0 = np.zeros(W_PER_CORE + 1, np.int64)
    for wi in range(W_PER_CORE):
        win_tile0[wi + 1] = win_tile0[wi] + len(win_tiles[wi])

    # ---- per-core data --------------------------------------------------
    # all pad slots (segment padding and call tails) use granule 0 so
    # every gt slot is written (unwritten SBUF can hold NaN patterns that
    # poison 0*NaN in the scatter matmuls); their C rows are zero.
    idx_streams = np.zeros((N_CORES, NSLOT), np.int16)

    # C matrices: per core, directly in device layout [128, NTILES*128]
    d2 = (dinv_pad ** 2).reshape(N_CORES, W_PER_CORE, P)
    ar = np.arange(P)
    loop_tiles = win_tile0[:W_PER_CORE]
    loop_cols = (loop_tiles[:, None] * P + ar[None, :]).ravel()
    loop_rows = np.tile(ar, W_PER_CORE)
    Cw = np.zeros((N_CORES, P, NTILES * P), BF16)
    for k in range(N_CORES):
        Ck = np.zeros((P, NTILES * P), np.float32)
        Ck[loop_rows, loop_cols] = d2[k].ravel()
        for wi in range(W_PER_CORE):
            for p in range(NPAR):
                c = cnt[k, wi, p]
                if c == 0:
                    continue
                e0 = seg_start_edge[k, wi, p]
                sl = seg_slot[wi, p] + np.arange(c)
                idx_streams[k, sl] = ogidx[e0:e0 + c]
                jlo = seg_slot[wi, p] // P
                t_first = tile_of[(wi, p, jlo)]
                tt = t_first + (sl // P - jlo)
                Ck[sl % P, tt * P + odst[e0:e0 + c]] = onorm[e0:e0 + c]
        Cw[k] = Ck

    # wrap idx: position i -> [i % 16, i // 16], replicated over 8 groups
    idxw = np.tile(
        idx_streams.reshape(N_CORES, NSLOT // 16, 16).transpose(0, 2, 1),
        (1, 8, 1))

    meta = dict(
        NBLK=NBLK, NSLOT=NSLOT, NTILES=NTILES,
        segsz=segsz, seg_slot=seg_slot,
        call_nblk=call_nblk, call_blk0=call_blk0,
        win_tiles=win_tiles, win_tile0=win_tile0,
    )
    return meta, idxw, Cw, dinv_pad


def _prep_nodes(x, batch, dinv_pad):
    xs = np.zeros((NODES_PAD, F_IN), np.float32)
    xs[:N_NODES] = x
    x_own = (xs.reshape(N_CORES, W_PER_CORE, P, F_IN)
             .transpose(0, 2, 1, 3)
             .reshape(N_CORES, P, W_PER_CORE * F_IN).copy())
    bf = np.full((NODES_PAD,), -1.0, np.float32)
    bf[:N_NODES] = batch.astype(np.float32)
    batchf = bf.reshape(N_CORES, W_PER_CORE, P).transpose(0, 2, 1).copy()
    dinvf = dinv_pad.reshape(N_CORES, W_PER_CORE, P).transpose(0, 2, 1).copy()
    return x_own, batchf, dinvf


# ------------------------------------------------------------- bass builder
def _build_nc(meta):
    import concourse.bacc as bacc
    import concourse.mybir as mybir
    import concourse.tile as tile
    from concourse.masks import make_identity

    f32 = mybir.dt.float32
    bf16 = mybir.dt.bfloat16
    i16 = mybir.dt.int16
    i32 = mybir.dt.int32
    AF = mybir.ActivationFunctionType
    OP = mybir.AluOpType

    NBLK = meta["NBLK"]
    NSLOT = meta["NSLOT"]
    NTILES = meta["NTILES"]
    call_nblk = meta["call_nblk"]
    call_blk0 = meta["call_blk0"]
    win_tiles = meta["win_tiles"]
    win_tile0 = meta["win_tile0"]
    NBLKG_MAX = int(call_nblk.sum(axis=1).max())
    NTW_MAX = max(len(t) for t in win_tiles)

    nc = bacc.Bacc("TRN2", target_bir_lowering=False, debug=False,
                   num_devices=N_CORES, num_swdge_queues=4)

    # ------------------------------------------------- I/O declarations
    x_own_t = nc.dram_tensor("x_own", [P, W_PER_CORE * F_IN], bf16,
                             kind="ExternalInput")
    idx_t = nc.dram_tensor("idxw", [P, NSLOT // 16], i16,
                           kind="ExternalInput")
    C_t = nc.dram_tensor("Cw", [P, NTILES * P], bf16, kind="ExternalInput")
    batch_t = nc.dram_tensor("batchf", [P, W_PER_CORE], f32,
                             kind="ExternalInput")
    dinv_t = nc.dram_tensor("dinvf", [P, W_PER_CORE], f32,
                            kind="ExternalInput")
    W1_t = nc.dram_tensor("W1b", [F_IN, 64], bf16, kind="ExternalInput")
    W2_t = nc.dram_tensor("W2b", [64, 128], bf16, kind="ExternalInput")
    W3_t = nc.dram_tensor("W3b", [128, 64], bf16, kind="ExternalInput")
    Wfc_t = nc.dram_tensor("Wfc", [64, 1], f32, kind="ExternalInput")
    b1_t = nc.dram_tensor("b1c", [64, 1], f32, kind="ExternalInput")
    b2_t = nc.dram_tensor("b2c", [128, 1], f32, kind="ExternalInput")
    b3_t = nc.dram_tensor("b3c", [64, 1], f32, kind="ExternalInput")
    bfc_t = nc.dram_tensor("bfcr", [64, 1], f32, kind="ExternalInput")
    T1_t = nc.dram_tensor("T1g", [NGRAN, 2 * P], bf16, kind="ExternalInput")
    out_t = nc.dram_tensor("out", [64, 1], f32, kind="ExternalOutput")

    RG = [list(range(N_CORES))]

    with tile.TileContext(nc) as tc:
        with (
            tc.tile_pool(name="dram", bufs=1, space="DRAM") as dram,
            tc.tile_pool(name="const", bufs=1) as const,
            tc.tile_pool(name="gat", bufs=3) as gpool,
            tc.tile_pool(name="cst", bufs=4) as cpool,
            tc.tile_pool(name="epi", bufs=2) as epool,
            tc.tile_pool(name="zps", bufs=2, space="PSUM") as zpool,
            tc.tile_pool(name="eps", bufs=2, space="PSUM") as eppool,
            tc.tile_pool(name="tps", bufs=1, space="PSUM") as tppool,
            tc.tile_pool(name="pps", bufs=1, space="PSUM") as ppool,
        ):
            # DRAM buffers: tables as granule views [NGRAN, 256 bf16]
            T_2 = dram.tile([NGRAN, 2 * P], bf16, addr_space="Shared")
            T_3 = dram.tile([NGRAN, 2 * P], bf16, addr_space="Shared")
            ag = dram.tile([NPC, FD], bf16)
            poolin = dram.tile([64, 65], f32)
            poolred = dram.tile([64, 65], f32, addr_space="Shared")

            # ------------------------------------------------- constants
            sid = const.tile([P, NSLOT // 16], i16)
            nc.sync.dma_start(out=sid[:], in_=idx_t[:])

            sbatch = const.tile([P, W_PER_CORE], f32)
            nc.sync.dma_start(out=sbatch[:], in_=batch_t[:])
            sdinv = const.tile([P, W_PER_CORE], f32)
            nc.sync.dma_start(out=sdinv[:], in_=dinv_t[:])
            sW1 = const.tile([F_IN, 64], bf16)
            nc.sync.dma_start(out=sW1[:], in_=W1_t[:])
            sW2 = const.tile([64, 128], bf16)
            nc.sync.dma_start(out=sW2[:], in_=W2_t[:])
            sW3 = const.tile([128, 64], bf16)
            nc.sync.dma_start(out=sW3[:], in_=W3_t[:])
            sWfc = const.tile([64, 1], f32)
            nc.sync.dma_start(out=sWfc[:], in_=Wfc_t[:])
            sb1 = const.tile([64, 1], f32)
            nc.sync.dma_start(out=sb1[:], in_=b1_t[:])
            sb2 = const.tile([128, 1], f32)
            nc.sync.dma_start(out=sb2[:], in_=b2_t[:])
            sb3 = const.tile([64, 1], f32)
            nc.sync.dma_start(out=sb3[:], in_=b3_t[:])
            sbfc = const.tile([64, 1], f32)
            nc.sync.dma_start(out=sbfc[:], in_=bfc_t[:])

            identf = const.tile([P, P], f32)
            make_identity(nc, identf[:])
            identb = const.tile([P, P], bf16)
            nc.vector.tensor_copy(out=identb[:], in_=identf[:])
            iog_i = const.tile([P, 64], i32)
            nc.gpsimd.iota(iog_i[:], pattern=[[1, 64]], channel_multiplier=0)
            iog_f = const.tile([P, 64], f32)
            nc.vector.tensor_copy(out=iog_f[:], in_=iog_i[:])
            iog_b = const.tile([P, 64], bf16)
            nc.vector.tensor_copy(out=iog_b[:], in_=iog_f[:])
            sbatch_b = const.tile([P, W_PER_CORE], bf16)
            nc.vector.tensor_copy(out=sbatch_b[:], in_=sbatch[:])

            Town = const.tile([P, W_PER_CORE * FD], bf16)

            # ------------------------------------------------- T1 build
            nc.vector.memset(Town[:], 0.0)
            nc.sync.dma_start(
                out=Town[:].rearrange("p (w f) -> p w f", f=FD)[:, :, 0:F_IN],
                in_=x_own_t[:].rearrange("p (w f) -> p w f", f=F_IN))


            pool_ps = ppool.tile([P, 512], f32, tag="pool")

            # ------------------------------------------------- layer loop
            def layer(lnum, T_src, FW):
                """lnum in {1,2,3}; FW = table payload width (16 or 64)."""
                for g in range(N_GROUPS):
                    gt = gpool.tile([P, NBLKG_MAX, P], bf16, tag="g")
                    goff = {}
                    off = 0
                    for p in range(NPAR):
                        nb = int(call_nblk[g, p])
                        goff[p] = (off, int(call_blk0[g, p]))
                        off += nb
                    # queues 1-3 are fire-and-forget (their Q7 pairs run
                    # concurrently); queue 0 blocks the engine, issue last
                    for p in (1, 2, 3, 0):
                        nb = int(call_nblk[g, p])
                        poff, b0 = goff[p]
                        nc.gpsimd.dma_gather(
                            out_ap=gt[:, poff:poff + nb, :],
                            in_ap=T_src[:, (p // 2) * P:(p // 2 + 1) * P],
                            idxs_ap=sid[:, b0 * 8:(b0 + nb) * 8],
                            num_idxs=nb * P, num_idxs_reg=nb * P,
                            elem_size=P, elem_step=2 * P,
                            single_packet=False, queue_num=(p + 1) % 4)
                    for w in range(g * WG, (g + 1) * WG):
                        ntw = len(win_tiles[w])
                        t0 = int(win_tile0[w])
                        Cw = cpool.tile([P, NTW_MAX * P], bf16, tag="C")
                        nc.scalar.dma_start(
                            out=Cw[:, 0:ntw * P],
                            in_=C_t[:, t0 * P:(t0 + ntw) * P])
                        zt = zpool.tile([64, P], f32, tag="z")
                        for ti, (kind, p, j) in enumerate(win_tiles[w]):
                            if kind == "loop":
                                lhs = Town[:, w * FD:w * FD + FW]
                            else:
                                poff, pb0 = goff[p]
                                jl = poff + (j - pb0)
                                cb = (p % 2) * 64
                                lhs = gt[:, jl, cb:cb + FW]
                            nc.tensor.matmul(
                                out=zt[0:FW, :], lhsT=lhs,
                                rhs=Cw[:, ti * P:(ti + 1) * P],
                                start=(ti == 0), stop=(ti == ntw - 1),
                                skip_group_check=True)
                        epilogue(lnum, w, zt)
                if lnum < 3:
                    T_dst = T_2 if lnum == 1 else T_3
                    nc.gpsimd.collective_compute(
                        "AllGather", OP.bypass, replica_groups=RG,
                        ins=[ag.opt()], outs=[T_dst.opt()])

            # ------------------------------------------------- epilogues
            def write_table(w, hT_sb):
                """hT_sb [64, 128] bf16 -> transpose -> Town + ag rows."""
                tp = tppool.tile([P, 64], bf16, tag="tp")
                nc.tensor.transpose(out=tp[:], in_=hT_sb,
                                    identity=identb[0:64, 0:64])
                nc.vector.tensor_copy(out=Town[:, w * FD:(w + 1) * FD],
                                      in_=tp[:])
                nc.sync.dma_start(
                    out=ag[w * P:(w + 1) * P, :],
                    in_=Town[:, w * FD:(w + 1) * FD])

            def epilogue(lnum, w, zt):
                if lnum == 1:
                    zb = epool.tile([F_IN, P], bf16, tag="zb1")
                    nc.vector.tensor_copy(out=zb[:], in_=zt[0:F_IN, :])
                    mm = eppool.tile([P, P], f32, tag="mm")
                    nc.tensor.matmul(out=mm[0:64, :], lhsT=sW1[:], rhs=zb[:],
                                     start=True, stop=True,
                                     skip_group_check=True)
                    hT = epool.tile([64, P], bf16, tag="h2T")
                    nc.scalar.activation(out=hT[:], in_=mm[0:64, :],
                                         func=AF.Relu, bias=sb1[:],
                                         scale=1.0)
                    write_table(w, hT[:])
                elif lnum == 2:
                    zb = epool.tile([64, P], bf16, tag="zb2")
                    nc.vector.tensor_copy(out=zb[:], in_=zt[0:64, :])
                    mm = eppool.tile([P, P], f32, tag="mm")
                    nc.tensor.matmul(out=mm[:], lhsT=sW2[:], rhs=zb[:],
                                     start=True, stop=True,
                                     skip_group_check=True)
                    h3 = epool.tile([P, P], bf16, tag="h3T")
                    nc.scalar.activation(out=h3[:], in_=mm[:],
                                         func=AF.Relu, bias=sb2[:],
                                         scale=1.0)
                    mm2 = eppool.tile([P, P], f32, tag="mm")
                    nc.tensor.matmul(out=mm2[0:64, :], lhsT=sW3[:],
                                     rhs=h3[:], start=True, stop=True,
                                     skip_group_check=True)
                    hT = epool.tile([64, P], bf16, tag="t3T")
                    nc.vector.tensor_copy(out=hT[:], in_=mm2[0:64, :])
                    write_table(w, hT[:])
                else:
                    hT = epool.tile([64, P], bf16, tag="h4T")
                    nc.scalar.activation(out=hT[:], in_=zt[0:64, :],
                                         func=AF.Relu, bias=sb3[:],
                                         scale=1.0)
                    tp = tppool.tile([P, 64], bf16, tag="tp")
                    nc.tensor.transpose(out=tp[:], in_=hT[:],
                                        identity=identb[0:64, 0:64])
                    hx = epool.tile([P, 65], bf16, tag="hx")
                    nc.vector.tensor_copy(out=hx[:, 0:64], in_=tp[:])
                    nc.vector.memset(hx[:, 64:65], 1.0)
                    S = epool.tile([P, 64], bf16, tag="S")
                    nc.vector.tensor_scalar(
                        out=S[:], in0=iog_b[:],
                        scalar1=sbatch[:, w:w + 1], scalar2=None,
                        op0=OP.is_equal)
                    nc.tensor.matmul(
                        out=pool_ps[0:64, 0:65], lhsT=S[:], rhs=hx[:],
                        start=(w == 0), stop=(w == W_PER_CORE - 1),
                        skip_group_check=True)

            layer(1, T1_t, F_IN)
            layer(2, T_2, FD)
            layer(3, T_3, FD)

            # ------------------------------------------------- pool + FC
            poolsb = epool.tile([64, 65], f32, tag="poolsb")
            nc.vector.tensor_copy(out=poolsb[:], in_=pool_ps[0:64, 0:65])
            nc.sync.dma_start(out=poolin[:], in_=poolsb[:])
            nc.gpsimd.collective_compute(
                "AllReduce", OP.add, replica_groups=RG,
                ins=[poolin.opt()], outs=[poolred.opt()])
            pr = epool.tile([64, 65], f32, tag="pr")
            nc.sync.dma_start(out=pr[:], in_=poolred[:])
            cntc = epool.tile([64, 1], f32, tag="cntc")
            nc.vector.tensor_scalar(out=cntc[:], in0=pr[:, 64:65],
                                    scalar1=1.0, scalar2=None, op0=OP.max)
            rcnt = epool.tile([64, 1], f32, tag="rcnt")
            nc.vector.reciprocal(out=rcnt[:], in_=cntc[:])
            mean = epool.tile([64, 64], f32, tag="mean")
            nc.vector.tensor_scalar(out=mean[:], in0=pr[:, 0:64],
                                    scalar1=rcnt[:], scalar2=None,
                                    op0=OP.mult)
            tpf = tppool.tile([P, P], f32, tag="tpf")
            nc.tensor.transpose(out=tpf[0:64, 0:64], in_=mean[:],
                                identity=identf[0:64, 0:64])
            meanT = epool.tile([64, 64], f32, tag="meanT")
            nc.vector.tensor_copy(out=meanT[:], in_=tpf[0:64, 0:64])
            op_ps = eppool.tile([P, P], f32, tag="mm")
            nc.tensor.matmul(out=op_ps[0:64, 0:1], lhsT=meanT[:],
                             rhs=sWfc[:], start=True, stop=True,
                             skip_group_check=True)
            ob = epool.tile([64, 1], f32, tag="ob")
            nc.vector.tensor_tensor(out=ob[:], in0=op_ps[0:64, 0:1],
                                    in1=sbfc[:], op=OP.add)
            nc.sync.dma_start(out=out_t[:], in_=ob[:])

    nc.finalize()
    return nc


# ------------------------------------------------------------------ runner
def _install_ntff_shim():
    try:
        import antenv
        if hasattr(antenv, "axon_hooks"):
            return
        mod = types.ModuleType("antenv.axon_hooks")
        mod._hook = None
        mod.set_axon_ntff_profile_hook = lambda h: setattr(mod, "_hook", h)
        mod.get_axon_ntff_profile_hook = lambda: mod._hook
        sys.modules["antenv.axon_hooks"] = mod
        antenv.axon_hooks = mod
        from trn_agent_boot.trn_boot import _ntff_profile_via_ctypes
        mod._hook = _ntff_profile_via_ctypes("/opt/axon/libaxon_pjrt.so")
    except Exception:
        pass


def kernel(x, edge_index, edge_weight, batch, W1, b1, W2, b2, W3, b3,
           Wfc, bfc):
    global LAST_EXEC_TIME_NS, LAST_TRACE, LAST_RESULT

    x = np.asarray(x, dtype=np.float32)
    ei = np.asarray(edge_index)
    src = ei[0].astype(np.int64)
    dst = ei[1].astype(np.int64)
    w = np.asarray(edge_weight, dtype=np.float32)
    batch = np.asarray(batch)

    meta, idxw, Cw, dinv_pad = _prep(src, dst, w)
    x_own, batchf, dinvf = _prep_nodes(x, batch, dinv_pad)
    x_own = x_own.astype(BF16)

    # T1 table in split-granule layout, directly from x
    xs = np.zeros((NODES_PAD, FD), np.float32)
    xs[:N_NODES, :F_IN] = x
    v = np.arange(NODES_PAD)
    gpos = v // 4
    lane = v % 4
    T1g = np.zeros((NGRAN, 4, FD), np.float32)
    T1g[gpos, lane, :] = xs
    T1g = T1g.reshape(NGRAN, 2 * P).astype(BF16)

    W1b = np.asarray(W1, np.float32).astype(BF16)
    W2b = np.asarray(W2, np.float32).astype(BF16)
    W3b = np.asarray(W3, np.float32).astype(BF16)
    Wfc = np.asarray(Wfc, np.float32).reshape(64, 1)
    b1c = np.asarray(b1, np.float32).reshape(64, 1)
    b2c = np.asarray(b2, np.float32).reshape(128, 1)
    b3c = np.asarray(b3, np.float32).reshape(64, 1)
    bfcr = np.tile(np.asarray(bfc, np.float32).reshape(1, 1), (64, 1))

    nc = _build_nc(meta)

    in_maps = []
    for k in range(N_CORES):
        in_maps.append({
            "x_own": x_own[k], "idxw": idxw[k], "Cw": Cw[k], "T1g": T1g,
            "batchf": batchf[k], "dinvf": dinvf[k],
            "W1b": W1b, "W2b": W2b, "W3b": W3b, "Wfc": Wfc,
            "b1c": b1c, "b2c": b2c, "b3c": b3c, "bfcr": bfcr,
        })

    trace = os.environ.get("BASS_GNN_TRACE", "") == "1"
    if trace:
        _install_ntff_shim()
        from concourse import bass_utils as _bu
        _bu.upload_artifacts = lambda tmpdir: tmpdir

    from concourse.bass_utils import run_bass_kernel_spmd
    res = run_bass_kernel_spmd(
        nc, in_maps, core_ids=list(range(N_CORES)), trace=trace,
    )
    LAST_RESULT = res
    if trace:
        LAST_EXEC_TIME_NS = res.exec_time_ns
        LAST_TRACE = (res.instructions_and_trace[1]
                      if res.instructions_and_trace else None)
    return np.asarray(res.results[0]["out"], dtype=np.float32)


# revision 3
# speedup vs baseline: 1.0221x; 1.0221x over previous
"""Trainium2 Bass kernel v2 for nn_EnhancedGNN (3-layer GCN + mean-pool + FC).

Design (dst-sharded, zT-oriented, host-baked normalization):
  - Tables are packed bf16 rows [node, 64] (128B). Gathers read 256B
    granules (2 packed rows) at stride 512B, giving 4 parity classes
    p = src % 4 with int16 granule index src // 4 (< 25088).
  - Slots are laid out uniformly across cores: per (window, parity)
    segment size = max over cores; per group (7 windows) one gather call
    per parity class, tail-padded with -1 (stripped, free).
  - Scatter = PE matmuls: out zT[f, 128 dst] += M_b.T @ C_tile, where
    M_b = gathered rows (lhsT, stationary) and C_tile [128, 128] bf16 is
    HOST-BAKED (value norm_e = dinv[src] w dinv[dst] at [slot, dstrel]).
    Self-loops are per-window diag(dinv^2) tiles against the resident
    own-table slice. deg/dinv all precomputed on host.
  - Epilogue per window works on zT [F, 128] in PSUM: W-matmuls with
    stationary weights, ACT relu+bias, PE transpose to row-major,
    write-back to the next layer's table; AllGather per layer; L3 pools
    via batch one-hot matmul; tiny AllReduce; FC on every core.
"""

import os
import sys
import types

import numpy as np

try:
    import ml_dtypes
    BF16 = ml_dtypes.bfloat16
except Exception:  # pragma: no cover
    BF16 = np.float32

# ---------------------------------------------------------------- constants
N_NODES = 100000
F_IN = 16
N_GRAPHS = 64
P = 128
N_CORES = 8
W_PER_CORE = 98
NPC = W_PER_CORE * P                 # 12544
NODES_PAD = N_CORES * NPC            # 100352
NPAR = 4                             # parity classes (stride 512B)
NGRAN = NODES_PAD // NPAR            # 25088 granules of 512B
WG = 7                               # windows per gather group
N_GROUPS = W_PER_CORE // WG          # 14
SPLIT_W = 70                         # windows in AllGather part A
SPLIT_ROWS = SPLIT_W * P             # 8960
FD = 64                              # table row payload (bf16)

LAST_EXEC_TIME_NS = None
LAST_TRACE = None
LAST_RESULT = None


# ---------------------------------------------------------------- host prep
def _prep(src, dst, w):
    """Builds uniform slot/tile structure + per-core data arrays."""
    E = src.shape[0]
    deg = np.bincount(dst, weights=w.astype(np.float64), minlength=N_NODES)
    deg += 1.0
    dinv = (1.0 / np.sqrt(deg)).astype(np.float32)
    dinv_pad = np.ones(NODES_PAD, np.float32)
    dinv_pad[:N_NODES] = dinv
    norm = dinv[src] * w.astype(np.float32) * dinv[dst]

    core = dst // NPC
    wl = (dst % NPC) // P
    dstrel = dst % P
    par = (src % NPAR).astype(np.int64)
    gidx = (src // NPAR).astype(np.int64)

    order = np.lexsort((src, par, wl, core))
    oc = core[order]
    owl = wl[order]
    opar = par[order]
    ogidx = gidx[order]
    odst = dstrel[order]
    onorm = norm[order]

    # per (core, w, p) counts
    key = (oc * W_PER_CORE + owl) * NPAR + opar
    cnt = np.bincount(key, minlength=N_CORES * W_PER_CORE * NPAR)
    cnt = cnt.reshape(N_CORES, W_PER_CORE, NPAR)
    segsz = cnt.max(axis=0)                      # [98, 4] uniform
    seg_start_edge = np.zeros((N_CORES, W_PER_CORE, NPAR), np.int64)
    flat = cnt.reshape(-1)
    seg_start_edge.reshape(-1)[1:] = np.cumsum(flat)[:-1]

    # ---- uniform slot layout -------------------------------------------
    # group g covers windows [g*WG, (g+1)*WG); call (g, p) concatenates
    # the group's p-segments, tail-padded to a block (128) multiple.
    seg_slot = np.zeros((W_PER_CORE, NPAR), np.int64)   # global slot of seg
    call_nblk = np.zeros((N_GROUPS, NPAR), np.int64)
    call_len = np.zeros((N_GROUPS, NPAR), np.int64)     # real idx count
    call_blk0 = np.zeros((N_GROUPS, NPAR), np.int64)    # first global block
    blk = 0
    for g in range(N_GROUPS):
        for p in range(NPAR):
            pos = 0
            for wi in range(g * WG, (g + 1) * WG):
                seg_slot[wi, p] = blk * P + pos
                pos += segsz[wi, p]
            call_len[g, p] = pos
            nb = -(-pos // P)
            call_nblk[g, p] = nb
            call_blk0[g, p] = blk
            blk += nb
    NBLK = blk
    NSLOT = NBLK * P

    # ---- tile structure (uniform) --------------------------------------
    # consumption: per window: [self-loop] + per parity: blocks jlo..jhi
    tile_of = {}                 # (w, p, jglobal) -> tile id
    win_tiles = []               # per window: list of (kind, p, jglobal)
    tid = 0
    for wi in range(W_PER_CORE):
        tl = [("loop", 0, 0)]
        tid += 1
        for p in range(NPAR):
            s0 = seg_slot[wi, p]
            c = segsz[wi, p]
            if c == 0:
                continue
            jlo, jhi = s0 // P, (s0 + c - 1) // P
            for j in range(jlo, jhi + 1):
                tile_of[(wi, p, j)] = tid
                tl.append(("seg", p, j))
                tid += 1
        win_tiles.append(tl)
    NTILES = tid
    win_tile0 = np.zeros(W_PER_CORE + 1, np.int64)
    for wi in range(W_PER_CORE):
        win_tile0[wi + 1] = win_tile0[wi] + len(win_tiles[wi])

    # ---- per-core data --------------------------------------------------
    # all pad slots (segment padding and call tails) use granule 0 so
    # every gt slot is written (unwritten SBUF can hold NaN patterns that
    # poison 0*NaN in the scatter matmuls); their C rows are zero.
    idx_streams = np.zeros((N_CORES, NSLOT), np.int16)

    # C matrices: per core, directly in device layout [128, NTILES*128]
    d2 = (dinv_pad ** 2).reshape(N_CORES, W_PER_CORE, P)
    ar = np.arange(P)
    loop_tiles = win_tile0[:W_PER_CORE]
    loop_cols = (loop_tiles[:, None] * P + ar[None, :]).ravel()
    loop_rows = np.tile(ar, W_PER_CORE)
    Cw = np.zeros((N_CORES, P, NTILES * P), BF16)
    for k in range(N_CORES):
        Ck = np.zeros((P, NTILES * P), np.float32)
        Ck[loop_rows, loop_cols] = d2[k].ravel()
        for wi in range(W_PER_CORE):
            for p in range(NPAR):
                c = cnt[k, wi, p]
                if c == 0:
                    continue
                e0 = seg_start_edge[k, wi, p]
                sl = seg_slot[wi, p] + np.arange(c)
                idx_streams[k, sl] = ogidx[e0:e0 + c]
                jlo = seg_slot[wi, p] // P
                t_first = tile_of[(wi, p, jlo)]
                tt = t_first + (sl // P - jlo)
                Ck[sl % P, tt * P + odst[e0:e0 + c]] = onorm[e0:e0 + c]
        Cw[k] = Ck

    # wrap idx: position i -> [i % 16, i // 16], replicated over 8 groups
    idxw = np.tile(
        idx_streams.reshape(N_CORES, NSLOT // 16, 16).transpose(0, 2, 1),
        (1, 8, 1))

    meta = dict(
        NBLK=NBLK, NSLOT=NSLOT, NTILES=NTILES,
        segsz=segsz, seg_slot=seg_slot,
        call_nblk=call_nblk, call_blk0=call_blk0,
        win_tiles=win_tiles, win_tile0=win_tile0,
    )
    return meta, idxw, Cw, dinv_pad


def _prep_nodes(x, batch, dinv_pad):
    xs = np.zeros((NODES_PAD, F_IN), np.float32)
    xs[:N_NODES] = x
    x_own = (xs.reshape(N_CORES, W_PER_CORE, P, F_IN)
             .transpose(0, 2, 1, 3)
             .reshape(N_CORES, P, W_PER_CORE * F_IN).copy())
    bf = np.full((NODES_PAD,), -1.0, np.float32)
    bf[:N_NODES] = batch.astype(np.float32)
    batchf = bf.reshape(N_CORES, W_PER_CORE, P).transpose(0, 2, 1).copy()
    dinvf = dinv_pad.reshape(N_CORES, W_PER_CORE, P).transpose(0, 2, 1).copy()
    return x_own, batchf, dinvf


# ------------------------------------------------------------- bass builder
def _build_nc(meta):
    import concourse.bacc as bacc
    import concourse.mybir as mybir
    import concourse.tile as tile
    from concourse.masks import make_identity

    f32 = mybir.dt.float32
    bf16 = mybir.dt.bfloat16
    i16 = mybir.dt.int16
    i32 = mybir.dt.int32
    AF = mybir.ActivationFunctionType
    OP = mybir.AluOpType

    NBLK = meta["NBLK"]
    NSLOT = meta["NSLOT"]
    NTILES = meta["NTILES"]
    call_nblk = meta["call_nblk"]
    call_blk0 = meta["call_blk0"]
    win_tiles = meta["win_tiles"]
    win_tile0 = meta["win_tile0"]
    NBLKG_MAX = int(call_nblk.sum(axis=1).max())
    NTW_MAX = max(len(t) for t in win_tiles)

    nc = bacc.Bacc("TRN2", target_bir_lowering=False, debug=False,
                   num_devices=N_CORES, num_swdge_queues=4)

    # ------------------------------------------------- I/O declarations
    x_own_t = nc.dram_tensor("x_own", [P, W_PER_CORE * F_IN], bf16,
                             kind="ExternalInput")
    idx_t = nc.dram_tensor("idxw", [P, NSLOT // 16], i16,
                           kind="ExternalInput")
    C_t = nc.dram_tensor("Cw", [P, NTILES * P], bf16, kind="ExternalInput")
    batch_t = nc.dram_tensor("batchf", [P, W_PER_CORE], f32,
                             kind="ExternalInput")
    dinv_t = nc.dram_tensor("dinvf", [P, W_PER_CORE], f32,
                            kind="ExternalInput")
    W1_t = nc.dram_tensor("W1b", [F_IN, 64], bf16, kind="ExternalInput")
    W2_t = nc.dram_tensor("W2b", [64, 128], bf16, kind="ExternalInput")
    W3_t = nc.dram_tensor("W3b", [128, 64], bf16, kind="ExternalInput")
    Wfc_t = nc.dram_tensor("Wfc", [64, 1], f32, kind="ExternalInput")
    b1_t = nc.dram_tensor("b1c", [64, 1], f32, kind="ExternalInput")
    b2_t = nc.dram_tensor("b2c", [128, 1], f32, kind="ExternalInput")
    b3_t = nc.dram_tensor("b3c", [64, 1], f32, kind="ExternalInput")
    bfc_t = nc.dram_tensor("bfcr", [64, 1], f32, kind="ExternalInput")
    T1_t = nc.dram_tensor("T1g", [NGRAN, 2 * P], bf16, kind="ExternalInput")
    out_t = nc.dram_tensor("out", [64, 1], f32, kind="ExternalOutput")

    RG = [list(range(N_CORES))]

    with tile.TileContext(nc) as tc:
        with (
            tc.tile_pool(name="dram", bufs=1, space="DRAM") as dram,
            tc.tile_pool(name="const", bufs=1) as const,
            tc.tile_pool(name="gat", bufs=3) as gpool,
            tc.tile_pool(name="cst", bufs=4) as cpool,
            tc.tile_pool(name="epi", bufs=2) as epool,
            tc.tile_pool(name="zps", bufs=3, space="PSUM") as zpool,
            tc.tile_pool(name="eps", bufs=2, space="PSUM") as eppool,
            tc.tile_pool(name="tps", bufs=1, space="PSUM") as tppool,
            tc.tile_pool(name="pps", bufs=1, space="PSUM") as ppool,
        ):
            # DRAM buffers: tables as granule views [NGRAN, 256 bf16]
            T_2 = dram.tile([NGRAN, 2 * P], bf16, addr_space="Shared")
            T_3 = dram.tile([NGRAN, 2 * P], bf16, addr_space="Shared")
            ag = dram.tile([NPC, FD], bf16)
            poolin = dram.tile([64, 65], f32)
            poolred = dram.tile([64, 65], f32, addr_space="Shared")

            # ------------------------------------------------- constants
            sid = const.tile([P, NSLOT // 16], i16)
            nc.sync.dma_start(out=sid[:], in_=idx_t[:])

            sbatch = const.tile([P, W_PER_CORE], f32)
            nc.sync.dma_start(out=sbatch[:], in_=batch_t[:])
            sdinv = const.tile([P, W_PER_CORE], f32)
            nc.sync.dma_start(out=sdinv[:], in_=dinv_t[:])
            sW1 = const.tile([F_IN, 64], bf16)
            nc.sync.dma_start(out=sW1[:], in_=W1_t[:])
            sW2 = const.tile([64, 128], bf16)
            nc.sync.dma_start(out=sW2[:], in_=W2_t[:])
            sW3 = const.tile([128, 64], bf16)
            nc.sync.dma_start(out=sW3[:], in_=W3_t[:])
            sWfc = const.tile([64, 1], f32)
            nc.sync.dma_start(out=sWfc[:], in_=Wfc_t[:])
            sb1 = const.tile([64, 1], f32)
            nc.sync.dma_start(out=sb1[:], in_=b1_t[:])
            sb2 = const.tile([128, 1], f32)
            nc.sync.dma_start(out=sb2[:], in_=b2_t[:])
            sb3 = const.tile([64, 1], f32)
            nc.sync.dma_start(out=sb3[:], in_=b3_t[:])
            sbfc = const.tile([64, 1], f32)
            nc.sync.dma_start(out=sbfc[:], in_=bfc_t[:])

            identf = const.tile([P, P], f32)
            make_identity(nc, identf[:])
            identb = const.tile([P, P], bf16)
            nc.vector.tensor_copy(out=identb[:], in_=identf[:])
            iog_i = const.tile([P, 64], i32)
            nc.gpsimd.iota(iog_i[:], pattern=[[1, 64]], channel_multiplier=0)
            iog_f = const.tile([P, 64], f32)
            nc.vector.tensor_copy(out=iog_f[:], in_=iog_i[:])
            iog_b = const.tile([P, 64], bf16)
            nc.vector.tensor_copy(out=iog_b[:], in_=iog_f[:])
            sbatch_b = const.tile([P, W_PER_CORE], bf16)
            nc.vector.tensor_copy(out=sbatch_b[:], in_=sbatch[:])

            Town = const.tile([P, W_PER_CORE * FD], bf16)

            # ------------------------------------------------- T1 build
            nc.vector.memset(Town[:], 0.0)
            nc.sync.dma_start(
                out=Town[:].rearrange("p (w f) -> p w f", f=FD)[:, :, 0:F_IN],
                in_=x_own_t[:].rearrange("p (w f) -> p w f", f=F_IN))


            pool_ps = ppool.tile([P, 512], f32, tag="pool")

            # ------------------------------------------------- layer loop
            def layer(lnum, T_src, FW):
                """lnum in {1,2,3}; FW = table payload width (16 or 64)."""
                pend = []
                for g in range(N_GROUPS):
                    gt = gpool.tile([P, NBLKG_MAX, P], bf16, tag="g")
                    goff = {}
                    off = 0
                    for p in range(NPAR):
                        nb = int(call_nblk[g, p])
                        goff[p] = (off, int(call_blk0[g, p]))
                        off += nb
                    # queues 1-3 are fire-and-forget (their Q7 pairs run
                    # concurrently); queue 0 blocks the engine, issue last
                    for p in (1, 2, 3, 0):
                        nb = int(call_nblk[g, p])
                        poff, b0 = goff[p]
                        nc.gpsimd.dma_gather(
                            out_ap=gt[:, poff:poff + nb, :],
                            in_ap=T_src[:, (p // 2) * P:(p // 2 + 1) * P],
                            idxs_ap=sid[:, b0 * 8:(b0 + nb) * 8],
                            num_idxs=nb * P, num_idxs_reg=nb * P,
                            elem_size=P, elem_step=2 * P,
                            single_packet=False, queue_num=(p + 1) % 4)
                    for w in range(g * WG, (g + 1) * WG):
                        ntw = len(win_tiles[w])
                        t0 = int(win_tile0[w])
                        Cw = cpool.tile([P, NTW_MAX * P], bf16, tag="C")
                        nc.scalar.dma_start(
                            out=Cw[:, 0:ntw * P],
                            in_=C_t[:, t0 * P:(t0 + ntw) * P])
                        zt = zpool.tile([64, P], f32, tag="z")
                        for ti, (kind, p, j) in enumerate(win_tiles[w]):
                            if kind == "loop":
                                lhs = Town[:, w * FD:w * FD + FW]
                            else:
                                poff, pb0 = goff[p]
                                jl = poff + (j - pb0)
                                cb = (p % 2) * 64
                                lhs = gt[:, jl, cb:cb + FW]
                            nc.tensor.matmul(
                                out=zt[0:FW, :], lhsT=lhs,
                                rhs=Cw[:, ti * P:(ti + 1) * P],
                                start=(ti == 0), stop=(ti == ntw - 1),
                                skip_group_check=True)
                        # lag epilogues 2 windows so the in-order PE never
                        # stalls on the DVE/ACT round trips of fresh data
                        pend.append((w, zt))
                        if len(pend) == 3:
                            pw, pzt = pend.pop(0)
                            epilogue(lnum, pw, pzt)
                for pw, pzt in pend:
                    epilogue(lnum, pw, pzt)
                if lnum < 3:
                    T_dst = T_2 if lnum == 1 else T_3
                    nc.gpsimd.collective_compute(
                        "AllGather", OP.bypass, replica_groups=RG,
                        ins=[ag.opt()], outs=[T_dst.opt()])

            # ------------------------------------------------- epilogues
            def write_table(w, hT_sb):
                """hT_sb [64, 128] bf16 -> transpose -> Town + ag rows."""
                tp = tppool.tile([P, 64], bf16, tag="tp")
                nc.tensor.transpose(out=tp[:], in_=hT_sb,
                                    identity=identb[0:64, 0:64])
                nc.vector.tensor_copy(out=Town[:, w * FD:(w + 1) * FD],
                                      in_=tp[:])
                nc.sync.dma_start(
                    out=ag[w * P:(w + 1) * P, :],
                    in_=Town[:, w * FD:(w + 1) * FD])

            def epilogue(lnum, w, zt):
                if lnum == 1:
                    zb = epool.tile([F_IN, P], bf16, tag="zb1")
                    nc.vector.tensor_copy(out=zb[:], in_=zt[0:F_IN, :])
                    mm = eppool.tile([P, P], f32, tag="mm")
                    nc.tensor.matmul(out=mm[0:64, :], lhsT=sW1[:], rhs=zb[:],
                                     start=True, stop=True,
                                     skip_group_check=True)
                    hT = epool.tile([64, P], bf16, tag="h2T")
                    nc.scalar.activation(out=hT[:], in_=mm[0:64, :],
                                         func=AF.Relu, bias=sb1[:],
                                         scale=1.0)
                    write_table(w, hT[:])
                elif lnum == 2:
                    zb = epool.tile([64, P], bf16, tag="zb2")
                    nc.vector.tensor_copy(out=zb[:], in_=zt[0:64, :])
                    mm = eppool.tile([P, P], f32, tag="mm")
                    nc.tensor.matmul(out=mm[:], lhsT=sW2[:], rhs=zb[:],
                                     start=True, stop=True,
                                     skip_group_check=True)
                    h3 = epool.tile([P, P], bf16, tag="h3T")
                    nc.scalar.activation(out=h3[:], in_=mm[:],
                                         func=AF.Relu, bias=sb2[:],
                                         scale=1.0)
                    mm2 = eppool.tile([P, P], f32, tag="mm")
                    nc.tensor.matmul(out=mm2[0:64, :], lhsT=sW3[:],
                                     rhs=h3[:], start=True, stop=True,
                                     skip_group_check=True)
                    hT = epool.tile([64, P], bf16, tag="t3T")
                    nc.vector.tensor_copy(out=hT[:], in_=mm2[0:64, :])
                    write_table(w, hT[:])
                else:
                    hT = epool.tile([64, P], bf16, tag="h4T")
                    nc.scalar.activation(out=hT[:], in_=zt[0:64, :],
                                         func=AF.Relu, bias=sb3[:],
                                         scale=1.0)
                    tp = tppool.tile([P, 64], bf16, tag="tp")
                    nc.tensor.transpose(out=tp[:], in_=hT[:],
                                        identity=identb[0:64, 0:64])
                    hx = epool.tile([P, 65], bf16, tag="hx")
                    nc.vector.tensor_copy(out=hx[:, 0:64], in_=tp[:])
                    nc.vector.memset(hx[:, 64:65], 1.0)
                    S = epool.tile([P, 64], bf16, tag="S")
                    nc.vector.tensor_scalar(
                        out=S[:], in0=iog_b[:],
                        scalar1=sbatch[:, w:w + 1], scalar2=None,
                        op0=OP.is_equal)
                    nc.tensor.matmul(
                        out=pool_ps[0:64, 0:65], lhsT=S[:], rhs=hx[:],
                        start=(w == 0), stop=(w == W_PER_CORE - 1),
                        skip_group_check=True)

            layer(1, T1_t, F_IN)
            layer(2, T_2, FD)
            layer(3, T_3, FD)

            # ------------------------------------------------- pool + FC
            poolsb = epool.tile([64, 65], f32, tag="poolsb")
            nc.vector.tensor_copy(out=poolsb[:], in_=pool_ps[0:64, 0:65])
            nc.sync.dma_start(out=poolin[:], in_=poolsb[:])
            nc.gpsimd.collective_compute(
                "AllReduce", OP.add, replica_groups=RG,
                ins=[poolin.opt()], outs=[poolred.opt()])
            pr = epool.tile([64, 65], f32, tag="pr")
            nc.sync.dma_start(out=pr[:], in_=poolred[:])
            cntc = epool.tile([64, 1], f32, tag="cntc")
            nc.vector.tensor_scalar(out=cntc[:], in0=pr[:, 64:65],
                                    scalar1=1.0, scalar2=None, op0=OP.max)
            rcnt = epool.tile([64, 1], f32, tag="rcnt")
            nc.vector.reciprocal(out=rcnt[:], in_=cntc[:])
            mean = epool.tile([64, 64], f32, tag="mean")
            nc.vector.tensor_scalar(out=mean[:], in0=pr[:, 0:64],
                                    scalar1=rcnt[:], scalar2=None,
                                    op0=OP.mult)
            tpf = tppool.tile([P, P], f32, tag="tpf")
            nc.tensor.transpose(out=tpf[0:64, 0:64], in_=mean[:],
                                identity=identf[0:64, 0:64])
            meanT = epool.tile([64, 64], f32, tag="meanT")
            nc.vector.tensor_copy(out=meanT[:], in_=tpf[0:64, 0:64])
            op_ps = eppool.tile([P, P], f32, tag="mm")
            nc.tensor.matmul(out=op_ps[0:64, 0:1], lhsT=meanT[:],
                             rhs=sWfc[:], start=True, stop=True,
                             skip_group_check=True)
            ob = epool.tile([64, 1], f32, tag="ob")
            nc.vector.tensor_tensor(out=ob[:], in0=op_ps[0:64, 0:1],
                                    in1=sbfc[:], op=OP.add)
            nc.sync.dma_start(out=out_t[:], in_=ob[:])

    nc.finalize()
    return nc


# ------------------------------------------------------------------ runner
def _install_ntff_shim():
    try:
        import antenv
        if hasattr(antenv, "axon_hooks"):
            return
        mod = types.ModuleType("antenv.axon_hooks")
        mod._hook = None
        mod.set_axon_ntff_profile_hook = lambda h: setattr(mod, "_hook", h)
        mod.get_axon_ntff_profile_hook = lambda: mod._hook
        sys.modules["antenv.axon_hooks"] = mod
        antenv.axon_hooks = mod
        from trn_agent_boot.trn_boot import _ntff_profile_via_ctypes
        mod._hook = _ntff_profile_via_ctypes("/opt/axon/libaxon_pjrt.so")
    except Exception:
        pass


def kernel(x, edge_index, edge_weight, batch, W1, b1, W2, b2, W3, b3,
           Wfc, bfc):
    global LAST_EXEC_TIME_NS, LAST_TRACE, LAST_RESULT

    x = np.asarray(x, dtype=np.float32)
    ei = np.asarray(edge_index)
    src = ei[0].astype(np.int64)
    dst = ei[1].astype(np.int64)
    w = np.asarray(edge_weight, dtype=np.float32)
    batch = np.asarray(batch)

    meta, idxw, Cw, dinv_pad = _prep(src, dst, w)
    x_own, batchf, dinvf = _prep_nodes(x, batch, dinv_pad)
    x_own = x_own.astype(BF16)

    # T1 table in split-granule layout, directly from x
    xs = np.zeros((NODES_PAD, FD), np.float32)
    xs[:N_NODES, :F_IN] = x
    v = np.arange(NODES_PAD)
    gpos = v // 4
    lane = v % 4
    T1g = np.zeros((NGRAN, 4, FD), np.float32)
    T1g[gpos, lane, :] = xs
    T1g = T1g.reshape(NGRAN, 2 * P).astype(BF16)

    W1b = np.asarray(W1, np.float32).astype(BF16)
    W2b = np.asarray(W2, np.float32).astype(BF16)
    W3b = np.asarray(W3, np.float32).astype(BF16)
    Wfc = np.asarray(Wfc, np.float32).reshape(64, 1)
    b1c = np.asarray(b1, np.float32).reshape(64, 1)
    b2c = np.asarray(b2, np.float32).reshape(128, 1)
    b3c = np.asarray(b3, np.float32).reshape(64, 1)
    bfcr = np.tile(np.asarray(bfc, np.float32).reshape(1, 1), (64, 1))

    nc = _build_nc(meta)

    in_maps = []
    for k in range(N_CORES):
        in_maps.append({
            "x_own": x_own[k], "idxw": idxw[k], "Cw": Cw[k], "T1g": T1g,
            "batchf": batchf[k], "dinvf": dinvf[k],
            "W1b": W1b, "W2b": W2b, "W3b": W3b, "Wfc": Wfc,
            "b1c": b1c, "b2c": b2c, "b3c": b3c, "bfcr": bfcr,
        })

    trace = os.environ.get("BASS_GNN_TRACE", "") == "1"
    if trace:
        _install_ntff_shim()
        from concourse import bass_utils as _bu
        _bu.upload_artifacts = lambda tmpdir: tmpdir

    from concourse.bass_utils import run_bass_kernel_spmd
    res = run_bass_kernel_spmd(
        nc, in_maps, core_ids=list(range(N_CORES)), trace=trace,
    )
    LAST_RESULT = res
    if trace:
        LAST_EXEC_TIME_NS = res.exec_time_ns
        LAST_TRACE = (res.instructions_and_trace[1]
                      if res.instructions_and_trace else None)
    return np.asarray(res.results[0]["out"], dtype=np.float32)
